# revision 26
# baseline (speedup 1.0000x reference)
"""Self-contained 8-NeuronCore Trainium2 kernel for a 16-head MHA layer.

Problem (hardcoded): x [2, 2048, 1024] f32, torch-style Linear weights
Wq/Wk/Wv/Wo [1024, 1024] + biases. y = MHA(x) with 16 heads of dim 64.

Sharding: tensor-parallel over heads. Core c owns heads {2c, 2c+1}, i.e.
feature slice F = [128c, 128c+128). Per core:

  phase 1   qT/kT/vT = W[F] @ x.T + b   (bf16 matmuls, feature-major)
            vT is PE-transposed per 128-token block into the PV lhsT
            layout v_pv[t, h0] = [v|1], v_pv[t, h1] = [1|v]  (bf16);
            the asymmetric ones-column placement keeps both heads'
            normalize chains partition-aligned (no shift DMA).
  phase 2   per (batch, q-chunk): dual-head S.T matmuls (K=64 row-group
            packed), exp on ACT only (scale 1/8 fused, bf16 out), PV
            accumulation with the denominator riding as the extra
            lhsT column; DVE reciprocal + K=1 matmul partition-broadcast;
            per-chunk output projection, staged zT written with one
            batched DMA per 512-token chunk.
  engines   ACT = exp only; PSUM evacuations split DVE/Pool; attention
            for batch b's first q-chunk is emitted interleaved with
            phase-1 chunks so ACT starts ~6us into the kernel.

Host sums the 8 partial zT outputs, adds bo, transposes back.
"""

import numpy as np

import concourse.bass as bass
import concourse.tile as tile
from concourse import mybir
from concourse.bass_utils import run_bass_kernel_spmd
from concourse.masks import make_identity

# ---------------------------------------------------------------- constants
B = 2
NSEQ = 2048
NIN = 1024
H = 16
DH = 64
P = 128
NTOK = B * NSEQ            # 4096
KO = NIN // P              # 8 contraction chunks for the projections
NCORES = 8
TCH = 512                  # token chunk (psum bank, free dim)
QH = 1024                  # q columns per S.T psum tile / exp instruction
KT = NSEQ // P             # 16 key-token tiles per (batch, head)
NCH = NTOK // TCH          # 8 projection chunks
EXP_BUFS = 24              # live expS tiles

F32 = mybir.dt.float32
F32R = mybir.dt.float32r
BF16 = mybir.dt.bfloat16
I16 = mybir.dt.int16
AF = mybir.ActivationFunctionType
AL = mybir.AluOpType

_SCALE = 0.125             # 1/sqrt(DH)

# DVE Schraudolph exp: i16 = round(A*(s/8) + B) bitcast to bf16 ~= exp(s/8).
# B is centered (16256 - 7) so the +-3% mantissa-linearization ripple is
# zero-mean and mixes cleanly with exact-exp tiles in the same softmax.
_SCHRA_A = 184.6649652337873 * _SCALE
_SCHRA_B = 16249.0
# Fraction of key-tiles whose exp runs on DVE instead of ACT (engine
# balancing; error contribution validated numerically, see notes).
SCHRA_NUM = 10             # kt % 16 < SCHRA_NUM  ->  DVE Schraudolph


# ------------------------------------------------- walrus workaround (env)
# The walrus build in this environment rejects instructions carrying more
# than one semaphore wait ("Too many sync wait commands").
def _patched_drain_and_barrier(self, tick_clock, wait_clock):
    nc = self.nc
    drain_inst = nc.sync.drain()
    wait_clock.add_sem_waits(
        drain_inst.ins, tile.ScopedClock({None: tick_clock.global_clock})
    )
    si = drain_inst.ins.sync_info
    waits = list(si.on_wait) if si is not None else []
    if len(waits) > 1:
        drain_inst.ins.sync_info = mybir.SyncInfo(
            on_wait=[waits[0]], on_update=list(si.on_update)
        )
        for w in waits[1:]:
            extra = nc.sync.drain()
            extra.ins.sync_info = mybir.SyncInfo(on_wait=[w], on_update=[])
    nc.all_engine_barrier()
    popped = nc._tile_sem_poison_stack.pop()
    assert popped is self._sem_poison
    nc.clear_and_free_semaphores(list(self.sems.allocated().values()))
    nc.all_engine_barrier()


def _install_drain_patch():
    if getattr(tile.TileContext, "_drain_patch_installed", False):
        return
    tile.TileContext._drain_and_barrier = _patched_drain_and_barrier
    tile.TileContext._drain_patch_installed = True


def _split_multi_waits(nc):
    """Move extra semaphore waits onto same-engine NoOps placed immediately
    before the instruction (engines execute their stream in order)."""
    k = 0
    for f in nc.m.functions:
        for blk in f.blocks:
            lst = blk.instructions
            i = 0
            while i < len(lst):
                inst = lst[i]
                si = inst.sync_info
                waits = list(si.on_wait) if si is not None else []
                if len(waits) > 1:
                    for w in waits[:-1]:
                        nop = mybir.InstNoOp(
                            name=f"waitsplit-{k}", engine=inst.engine
                        )
                        k += 1
                        nop.sync_info = mybir.SyncInfo(on_wait=[w], on_update=[])
                        nc.register_instruction(nop)
                        lst.insert(i, nop)
                        i += 1
                    inst.sync_info = mybir.SyncInfo(
                        on_wait=[waits[-1]], on_update=list(si.on_update)
                    )
                i += 1
    return k


# ------------------------------------------------------------ device kernel
def _emit(tc, xT, wq, wk, wv, bqkv, wo, ones64, zT):
    nc = tc.nc

    xT_r = xT.rearrange("(ko ki) t -> ki ko t", ki=P)
    zT_r = zT.rearrange("(co ki) t -> ki co t", ki=P)

    with (
        tc.tile_pool(name="const", bufs=1) as const,
        tc.tile_pool(name="persist", bufs=1) as persist,
        tc.tile_pool(name="xin", bufs=2) as xin,
        tc.tile_pool(name="vtmp", bufs=2) as vtmp,
        tc.tile_pool(name="exps", bufs=EXP_BUFS) as exps,
        tc.tile_pool(name="work", bufs=4) as work,
        tc.tile_pool(name="zout", bufs=2) as zout,
        tc.tile_pool(name="ps_big", bufs=2, space="PSUM") as ps_big,
        tc.tile_pool(name="ps_pv", bufs=2, space="PSUM") as ps_pv,
        tc.tile_pool(name="ps_mm", bufs=2, space="PSUM") as ps_mm,
    ):
        # ---- constants
        wq_sb = const.tile([P, KO, P], BF16)
        wk_sb = const.tile([P, KO, P], BF16)
        wv_sb = const.tile([P, KO, P], BF16)
        for w_sb, w in ((wq_sb, wq), (wk_sb, wk), (wv_sb, wv)):
            nc.sync.dma_start(out=w_sb, in_=w.rearrange("(ko ki) m -> ki ko m", ki=P))
        wo_sb = const.tile([P, NIN], BF16)
        nc.sync.dma_start(out=wo_sb, in_=wo)
        bias_sb = const.tile([P, 3], F32)
        nc.sync.dma_start(out=bias_sb, in_=bqkv)
        ones_sb = const.tile([1, DH], F32R)
        nc.sync.dma_start(out=ones_sb, in_=ones64)
        ident = const.tile([P, P], BF16)
        make_identity(nc, ident)

        qT = persist.tile([P, NTOK], BF16)
        kT = persist.tile([P, NTOK], BF16)
        # PV lhsT per 128-token block, per head slot: [v(64) | 1]; the ones
        # column makes the softmax denominator ride as psum row 64.
        v_pv = persist.tile([P, NTOK // P, 2, DH + 1], BF16)
        nc.vector.memset(v_pv, 1.0)
        OT = persist.tile([P, NTOK], BF16)

        etiles = {}

        def emit_st_exp(b, j, kt):
            """Dual-head S.T matmul pair for one 128-key tile + fused exp.
            exp runs on ACT (exact) or DVE (Schraudolph int16 bitcast) so the
            two engines split the softmax elementwise load."""
            qoff = b * NSEQ + j * TCH
            ps = ps_big.tile([P, QH], F32, tag="st")
            ksl = slice(b * NSEQ + kt * P, b * NSEQ + (kt + 1) * P)
            for hl in range(2):
                hsl = slice(DH * hl, DH * hl + DH)
                nc.tensor.matmul(
                    ps[:, hl * TCH : (hl + 1) * TCH],
                    kT[hsl, ksl],
                    qT[hsl, qoff : qoff + TCH],
                    start=True,
                    stop=True,
                )
            if kt % KT < SCHRA_NUM:
                ei = exps.tile([P, QH], I16, tag="e")
                nc.vector.tensor_scalar(
                    out=ei, in0=ps, scalar1=_SCHRA_A, scalar2=_SCHRA_B,
                    op0=AL.mult, op1=AL.add,
                )
                e = ei.bitcast(BF16)
            else:
                e = exps.tile([P, QH], BF16, tag="e")
                nc.scalar.activation(e, ps, AF.Exp, scale=_SCALE)
            etiles[(b, j, kt)] = e

        def emit_chunk(n):
            """QKV projections for one 512-token chunk (feature-major)."""
            tsl = slice(n * TCH, (n + 1) * TCH)
            xt = xin.tile([P, KO, TCH], BF16, tag="xt")
            nc.sync.dma_start(out=xt, in_=xT_r[:, :, tsl])
            # q, k  (evacuation + bias on ACT; DVE is reserved for exp work)
            for pi, (w_sb, dst) in enumerate(((wq_sb, qT), (wk_sb, kT))):
                ps = ps_mm.tile([P, TCH], F32, tag="mm")
                for ko in range(KO):
                    nc.tensor.matmul(
                        ps, w_sb[:, ko], xt[:, ko],
                        start=(ko == 0), stop=(ko == KO - 1),
                    )
                nc.scalar.activation(
                    dst[:, tsl], ps, AF.Identity,
                    bias=bias_sb[:, pi : pi + 1],
                )
            # v: feature-major matmul + bias, then PE transpose per
            # 128-token block into the packed PV lhsT layout.
            ps = ps_mm.tile([P, TCH], F32, tag="mm")
            for ko in range(KO):
                nc.tensor.matmul(
                    ps, wv_sb[:, ko], xt[:, ko],
                    start=(ko == 0), stop=(ko == KO - 1),
                )
            vt = vtmp.tile([P, TCH], BF16, tag="vt")
            nc.scalar.activation(vt, ps, AF.Identity, bias=bias_sb[:, 2:3])
            pst = ps_mm.tile([P, TCH], BF16, tag="mm")
            for t2 in range(TCH // P):
                nc.tensor.matmul(
                    pst[:, t2 * P : (t2 + 1) * P],
                    vt[:, t2 * P : (t2 + 1) * P],
                    ident,
                    is_transpose=True,
                    start=True,
                    stop=True,
                )
            pst_v = pst.rearrange("p (t2 h f) -> p t2 h f", t2=TCH // P, h=2)
            t0 = n * (TCH // P)
            nc.scalar.activation(
                v_pv[:, t0 : t0 + TCH // P, :, 0:DH], pst_v, AF.Identity
            )

        def emit_attention_tail(b, j):
            """PV + normalize + output projection for one (batch, q-chunk).
            Assumes S.T/exp for (b, j, *) already emitted."""
            qoff = b * NSEQ + j * TCH
            dst = slice(qoff, qoff + TCH)
            pvps = {}
            for hl in (1, 0):
                pvps[hl] = ps_pv.tile(
                    [P, TCH], F32, tag="pv", name=f"pv_{b}_{j}_{hl}"
                )
            # Both heads accumulate on psum rows 0..64 ([v|1], denominator
            # at row 64); interleaved so the two normalize chains overlap.
            for kt in range(KT):
                e = etiles[(b, j, kt)]
                for hl in (1, 0):
                    nc.tensor.matmul(
                        pvps[hl][0 : DH + 1, :],
                        v_pv[:, b * KT + kt, hl],
                        e[:, hl * TCH : (hl + 1) * TCH],
                        start=(kt == 0),
                        stop=(kt == KT - 1),
                    )
            for hl in (1, 0):      # h1 first: its OT shift-DMA overlaps h0
                ps = pvps[hl]
                rec = work.tile([1, TCH], F32R, tag="rec")
                with nc.allow_low_precision(
                    reason="f32r is bit-identical to f32; PE rounds on read"
                ):
                    nc.vector.reciprocal(rec, ps[DH : DH + 1, :])
                # partition-broadcast via K=1 matmul
                psb = ps_mm.tile([P, TCH], F32, tag="mm")
                nc.tensor.matmul(
                    psb[0:DH, :], ones_sb, rec, start=True, stop=True
                )
                recB = work.tile([DH, TCH], F32, tag="recB")
                nc.scalar.activation(recB, psb[0:DH, :], AF.Identity)
                if hl == 0:
                    nc.vector.tensor_mul(OT[0:DH, dst], ps[0:DH, :], recB)
                else:
                    tmpO = work.tile([DH, TCH], BF16, tag="tmpO")
                    nc.vector.tensor_mul(tmpO, ps[0:DH, :], recB)
                    nc.sync.dma_start(out=OT[DH:P, dst], in_=tmpO)
            for kt in range(KT):
                del etiles[(b, j, kt)]
            # ---- output projection for this 512-token chunk
            zst = zout.tile([P, NIN // P, TCH], F32, tag="z")
            for co in range(NIN // P):
                pz = ps_mm.tile([P, TCH], F32, tag="mm")
                nc.tensor.matmul(
                    pz,
                    wo_sb[:, co * P : (co + 1) * P],
                    OT[:, qoff : qoff + TCH],
                    start=True,
                    stop=True,
                )
                nc.scalar.activation(zst[:, co], pz, AF.Identity)
            nc.sync.dma_start(out=zT_r[:, :, qoff : qoff + TCH], in_=zst)

        # ---- emission: per batch, QKV chunks with the first q-chunk's
        # S.T/exp interleaved (ACT starts early), then the attention tails.
        for b in range(B):
            for c in range(NCH // B):
                emit_chunk(b * (NCH // B) + c)
                for kt in range(4 * c, 4 * c + 4):
                    emit_st_exp(b, 0, kt)
            emit_attention_tail(b, 0)
            for j in range(1, NSEQ // TCH):
                for kt in range(KT):
                    emit_st_exp(b, j, kt)
                emit_attention_tail(b, j)


def _build_nc(repeat=1):
    _install_drain_patch()
    nc = bass.Bass("TRN2", target_bir_lowering=False, debug=False, num_devices=NCORES)
    xT = nc.dram_tensor("xT", [NIN, NTOK], BF16, kind="ExternalInput").ap()
    wq = nc.dram_tensor("wq", [NIN, P], BF16, kind="ExternalInput").ap()
    wk = nc.dram_tensor("wk", [NIN, P], BF16, kind="ExternalInput").ap()
    wv = nc.dram_tensor("wv", [NIN, P], BF16, kind="ExternalInput").ap()
    bqkv = nc.dram_tensor("bqkv", [P, 3], F32, kind="ExternalInput").ap()
    wo = nc.dram_tensor("wo", [P, NIN], BF16, kind="ExternalInput").ap()
    ones64 = nc.dram_tensor("ones64", [1, DH], F32R, kind="ExternalInput").ap()
    zT = nc.dram_tensor("zT", [NIN, NTOK], F32, kind="ExternalOutput").ap()
    with tile.TileContext(nc, num_cores=NCORES) as tc:
        if repeat == 1:
            _emit(tc, xT, wq, wk, wv, bqkv, wo, ones64, zT)
        else:
            with tc.For_i(0, repeat):
                _emit(tc, xT, wq, wk, wv, bqkv, wo, ones64, zT)
    _split_multi_waits(nc)
    return nc


_NC_CACHE = None


def _get_nc():
    global _NC_CACHE
    if _NC_CACHE is None:
        _NC_CACHE = _build_nc()
    return _NC_CACHE


# -------------------------------------------------------------- host wrapper
def _to_bf16(a):
    import ml_dtypes

    return np.ascontiguousarray(a).astype(ml_dtypes.bfloat16)


def _in_maps(x, Wq, bq, Wk, bk, Wv, bv, Wo):
    xTh = _to_bf16(x.reshape(NTOK, NIN).T)
    maps = []
    for c in range(NCORES):
        F = slice(P * c, P * (c + 1))
        maps.append(
            {
                "xT": xTh,
                "wq": _to_bf16(Wq[F].T),
                "wk": _to_bf16(Wk[F].T),
                "wv": _to_bf16(Wv[F].T),
                "bqkv": np.ascontiguousarray(
                    np.stack([bq[F], bk[F], bv[F]], axis=1).astype(np.float32)
                ),
                "wo": _to_bf16(Wo[:, F].T),
                "ones64": np.ones((1, DH), np.float32),
            }
        )
    return maps


def kernel(x, Wq, bq, Wk, bk, Wv, bv, Wo, bo, **run_kwargs):
    x = np.asarray(x, np.float32)
    maps = _in_maps(
        x,
        np.asarray(Wq, np.float32),
        np.asarray(bq, np.float32),
        np.asarray(Wk, np.float32),
        np.asarray(bk, np.float32),
        np.asarray(Wv, np.float32),
        np.asarray(bv, np.float32),
        np.asarray(Wo, np.float32),
    )
    nc = _get_nc()
    res = run_bass_kernel_spmd(nc, maps, list(range(NCORES)), **run_kwargs)
    acc = res.results[0]["zT"].astype(np.float32)
    for c in range(1, NCORES):
        acc = acc + res.results[c]["zT"]
    z = acc.T + np.asarray(bo, np.float32)[None, :]
    out = np.ascontiguousarray(z.reshape(B, NSEQ, NIN), dtype=np.float32)
    if run_kwargs:
        return out, res
    return out


# revision 28
# speedup vs baseline: 1.3768x; 1.3768x over previous
"""Self-contained 8-NeuronCore Trainium2 kernel for a 16-head MHA layer.

Problem (hardcoded): x [2, 2048, 1024] f32, torch-style Linear weights
Wq/Wk/Wv/Wo [1024, 1024] + biases. y = MHA(x) with 16 heads of dim 64.

Sharding: tensor-parallel over heads. Core c owns heads {2c, 2c+1}, i.e.
feature slice F = [128c, 128c+128). Per core:

  phase 1   qT/kT/vT = W[F] @ x.T + b   (bf16 matmuls, feature-major)
            vT is PE-transposed per 128-token block into the PV lhsT
            layout v_pv[t, h0] = [v|1], v_pv[t, h1] = [1|v]  (bf16);
            the asymmetric ones-column placement keeps both heads'
            normalize chains partition-aligned (no shift DMA).
  phase 2   per (batch, q-chunk): dual-head S.T matmuls (K=64 row-group
            packed), exp on ACT only (scale 1/8 fused, bf16 out), PV
            accumulation with the denominator riding as the extra
            lhsT column; DVE reciprocal + K=1 matmul partition-broadcast;
            per-chunk output projection, staged zT written with one
            batched DMA per 512-token chunk.
  engines   ACT = exp only; PSUM evacuations split DVE/Pool; attention
            for batch b's first q-chunk is emitted interleaved with
            phase-1 chunks so ACT starts ~6us into the kernel.

Host sums the 8 partial zT outputs, adds bo, transposes back.
"""

import numpy as np

import concourse.bass as bass
import concourse.tile as tile
from concourse import mybir
from concourse.bass_utils import run_bass_kernel_spmd
from concourse.masks import make_identity

# ---------------------------------------------------------------- constants
B = 2
NSEQ = 2048
NIN = 1024
H = 16
DH = 64
P = 128
NTOK = B * NSEQ            # 4096
KO = NIN // P              # 8 contraction chunks for the projections
NCORES = 8
TCH = 512                  # token chunk (psum bank, free dim)
QH = 1024                  # q columns per S.T psum tile / exp instruction
KT = NSEQ // P             # 16 key-token tiles per (batch, head)
NCH = NTOK // TCH          # 8 projection chunks
EXP_BUFS = 24              # live expS tiles

F32 = mybir.dt.float32
F32R = mybir.dt.float32r
BF16 = mybir.dt.bfloat16
I16 = mybir.dt.int16
AF = mybir.ActivationFunctionType
AL = mybir.AluOpType

_SCALE = 0.125             # 1/sqrt(DH)

# DVE Schraudolph exp: i16 = round(A*(s/8) + B) bitcast to bf16 ~= exp(s/8).
# B is centered (16256 - 7) so the +-3% mantissa-linearization ripple is
# zero-mean and mixes cleanly with exact-exp tiles in the same softmax.
_SCHRA_A = 184.6649652337873 * _SCALE
_SCHRA_B = 16249.0
# Fraction of key-tiles whose exp runs on DVE instead of ACT (engine
# balancing; error contribution validated numerically, see notes).
SCHRA_NUM = 10             # kt % 16 < SCHRA_NUM  ->  DVE Schraudolph


# ------------------------------------------------- walrus workaround (env)
# The walrus build in this environment rejects instructions carrying more
# than one semaphore wait ("Too many sync wait commands").
def _patched_drain_and_barrier(self, tick_clock, wait_clock):
    nc = self.nc
    drain_inst = nc.sync.drain()
    wait_clock.add_sem_waits(
        drain_inst.ins, tile.ScopedClock({None: tick_clock.global_clock})
    )
    si = drain_inst.ins.sync_info
    waits = list(si.on_wait) if si is not None else []
    if len(waits) > 1:
        drain_inst.ins.sync_info = mybir.SyncInfo(
            on_wait=[waits[0]], on_update=list(si.on_update)
        )
        for w in waits[1:]:
            extra = nc.sync.drain()
            extra.ins.sync_info = mybir.SyncInfo(on_wait=[w], on_update=[])
    nc.all_engine_barrier()
    popped = nc._tile_sem_poison_stack.pop()
    assert popped is self._sem_poison
    nc.clear_and_free_semaphores(list(self.sems.allocated().values()))
    nc.all_engine_barrier()


def _install_drain_patch():
    if getattr(tile.TileContext, "_drain_patch_installed", False):
        return
    tile.TileContext._drain_and_barrier = _patched_drain_and_barrier
    tile.TileContext._drain_patch_installed = True


def _split_multi_waits(nc):
    """Move extra semaphore waits onto same-engine NoOps placed immediately
    before the instruction (engines execute their stream in order)."""
    k = 0
    for f in nc.m.functions:
        for blk in f.blocks:
            lst = blk.instructions
            i = 0
            while i < len(lst):
                inst = lst[i]
                si = inst.sync_info
                waits = list(si.on_wait) if si is not None else []
                if len(waits) > 1:
                    for w in waits[:-1]:
                        nop = mybir.InstNoOp(
                            name=f"waitsplit-{k}", engine=inst.engine
                        )
                        k += 1
                        nop.sync_info = mybir.SyncInfo(on_wait=[w], on_update=[])
                        nc.register_instruction(nop)
                        lst.insert(i, nop)
                        i += 1
                    inst.sync_info = mybir.SyncInfo(
                        on_wait=[waits[-1]], on_update=list(si.on_update)
                    )
                i += 1
    return k


# ------------------------------------------------------------ device kernel
def _emit(tc, xT, wq, wk, wv, bqkv, wo, ones64, zT):
    nc = tc.nc

    xT_r = xT.rearrange("(ko ki) t -> ki ko t", ki=P)
    zT_r = zT.rearrange("(co ki) t -> ki co t", ki=P)

    with (
        tc.tile_pool(name="const", bufs=1) as const,
        tc.tile_pool(name="persist", bufs=1) as persist,
        tc.tile_pool(name="xin", bufs=2) as xin,
        tc.tile_pool(name="vtmp", bufs=2) as vtmp,
        tc.tile_pool(name="exps", bufs=EXP_BUFS) as exps,
        tc.tile_pool(name="work", bufs=4) as work,
        tc.tile_pool(name="zout", bufs=2) as zout,
        tc.tile_pool(name="ps_big", bufs=2, space="PSUM") as ps_big,
        tc.tile_pool(name="ps_pv", bufs=2, space="PSUM") as ps_pv,
        tc.tile_pool(name="ps_mm", bufs=2, space="PSUM") as ps_mm,
    ):
        # ---- constants
        wq_sb = const.tile([P, KO, P], BF16)
        wk_sb = const.tile([P, KO, P], BF16)
        wv_sb = const.tile([P, KO, P], BF16)
        for w_sb, w in ((wq_sb, wq), (wk_sb, wk), (wv_sb, wv)):
            nc.sync.dma_start(out=w_sb, in_=w.rearrange("(ko ki) m -> ki ko m", ki=P))
        wo_sb = const.tile([P, NIN], BF16)
        nc.sync.dma_start(out=wo_sb, in_=wo)
        bias_sb = const.tile([P, 3], F32)
        nc.sync.dma_start(out=bias_sb, in_=bqkv)
        ones_sb = const.tile([1, DH], F32R)
        nc.sync.dma_start(out=ones_sb, in_=ones64)
        ident = const.tile([P, P], BF16)
        make_identity(nc, ident)

        qT = persist.tile([P, NTOK], BF16)
        kT = persist.tile([P, NTOK], BF16)
        # PV lhsT per 128-token block, per head slot: [v(64) | 1]; the ones
        # column makes the softmax denominator ride as psum row 64.
        v_pv = persist.tile([P, NTOK // P, 2, DH + 1], BF16)
        nc.vector.memset(v_pv, 1.0)
        OT = persist.tile([P, NTOK], BF16)

        etiles = {}

        def emit_st_exp(b, j, kt):
            """Dual-head S.T matmul pair for one 128-key tile + fused exp.
            exp runs on ACT (exact) or DVE (Schraudolph int16 bitcast) so the
            two engines split the softmax elementwise load."""
            qoff = b * NSEQ + j * TCH
            ps = ps_big.tile([P, QH], F32, tag="st")
            ksl = slice(b * NSEQ + kt * P, b * NSEQ + (kt + 1) * P)
            for hl in range(2):
                hsl = slice(DH * hl, DH * hl + DH)
                nc.tensor.matmul(
                    ps[:, hl * TCH : (hl + 1) * TCH],
                    kT[hsl, ksl],
                    qT[hsl, qoff : qoff + TCH],
                    start=True,
                    stop=True,
                )
            if kt % KT < SCHRA_NUM:
                ei = exps.tile([P, QH], I16, tag="e")
                nc.vector.tensor_scalar(
                    out=ei, in0=ps, scalar1=_SCHRA_A, scalar2=_SCHRA_B,
                    op0=AL.mult, op1=AL.add,
                )
                e = ei.bitcast(BF16)
            else:
                e = exps.tile([P, QH], BF16, tag="e")
                nc.scalar.activation(e, ps, AF.Exp, scale=_SCALE)
            etiles[(b, j, kt)] = e

        def emit_chunk(n):
            """QKV projections for one 512-token chunk (feature-major)."""
            tsl = slice(n * TCH, (n + 1) * TCH)
            xt = xin.tile([P, KO, TCH], BF16, tag="xt")
            nc.sync.dma_start(out=xt, in_=xT_r[:, :, tsl])
            # q, k  (evacuation + bias on ACT; DVE is reserved for exp work)
            for pi, (w_sb, dst) in enumerate(((wq_sb, qT), (wk_sb, kT))):
                ps = ps_mm.tile([P, TCH], F32, tag="mm")
                for ko in range(KO):
                    nc.tensor.matmul(
                        ps, w_sb[:, ko], xt[:, ko],
                        start=(ko == 0), stop=(ko == KO - 1),
                    )
                nc.scalar.activation(
                    dst[:, tsl], ps, AF.Identity,
                    bias=bias_sb[:, pi : pi + 1],
                )
            # v: feature-major matmul + bias, then PE transpose per
            # 128-token block into the packed PV lhsT layout.
            ps = ps_mm.tile([P, TCH], F32, tag="mm")
            for ko in range(KO):
                nc.tensor.matmul(
                    ps, wv_sb[:, ko], xt[:, ko],
                    start=(ko == 0), stop=(ko == KO - 1),
                )
            vt = vtmp.tile([P, TCH], BF16, tag="vt")
            nc.scalar.activation(vt, ps, AF.Identity, bias=bias_sb[:, 2:3])
            pst = ps_mm.tile([P, TCH], BF16, tag="mm")
            for t2 in range(TCH // P):
                nc.tensor.matmul(
                    pst[:, t2 * P : (t2 + 1) * P],
                    vt[:, t2 * P : (t2 + 1) * P],
                    ident,
                    is_transpose=True,
                    start=True,
                    stop=True,
                )
            pst_v = pst.rearrange("p (t2 h f) -> p t2 h f", t2=TCH // P, h=2)
            t0 = n * (TCH // P)
            nc.scalar.activation(
                v_pv[:, t0 : t0 + TCH // P, :, 0:DH], pst_v, AF.Identity
            )

        def emit_attention_tail(b, j):
            """PV + normalize + output projection for one (batch, q-chunk).
            Assumes S.T/exp for (b, j, *) already emitted."""
            qoff = b * NSEQ + j * TCH
            dst = slice(qoff, qoff + TCH)
            pvps = {}
            for hl in (1, 0):
                pvps[hl] = ps_pv.tile(
                    [P, TCH], F32, tag="pv", name=f"pv_{b}_{j}_{hl}"
                )
            # Both heads accumulate on psum rows 0..64 ([v|1], denominator
            # at row 64); interleaved so the two normalize chains overlap.
            for kt in range(KT):
                e = etiles[(b, j, kt)]
                for hl in (1, 0):
                    nc.tensor.matmul(
                        pvps[hl][0 : DH + 1, :],
                        v_pv[:, b * KT + kt, hl],
                        e[:, hl * TCH : (hl + 1) * TCH],
                        start=(kt == 0),
                        stop=(kt == KT - 1),
                    )
            for hl in (1, 0):      # h1 first: its OT shift-DMA overlaps h0
                ps = pvps[hl]
                rec = work.tile([1, TCH], F32R, tag="rec")
                with nc.allow_low_precision(
                    reason="f32r is bit-identical to f32; PE rounds on read"
                ):
                    nc.vector.reciprocal(rec, ps[DH : DH + 1, :])
                # partition-broadcast via K=1 matmul
                psb = ps_mm.tile([P, TCH], F32, tag="mm")
                nc.tensor.matmul(
                    psb[0:DH, :], ones_sb, rec, start=True, stop=True
                )
                recB = work.tile([DH, TCH], F32, tag="recB")
                nc.scalar.activation(recB, psb[0:DH, :], AF.Identity)
                if hl == 0:
                    nc.vector.tensor_mul(OT[0:DH, dst], ps[0:DH, :], recB)
                else:
                    tmpO = work.tile([DH, TCH], BF16, tag="tmpO")
                    nc.vector.tensor_mul(tmpO, ps[0:DH, :], recB)
                    nc.sync.dma_start(out=OT[DH:P, dst], in_=tmpO)
            for kt in range(KT):
                del etiles[(b, j, kt)]
            # ---- output projection for this 512-token chunk
            zst = zout.tile([P, NIN // P, TCH], F32, tag="z")
            for co in range(NIN // P):
                pz = ps_mm.tile([P, TCH], F32, tag="mm")
                nc.tensor.matmul(
                    pz,
                    wo_sb[:, co * P : (co + 1) * P],
                    OT[:, qoff : qoff + TCH],
                    start=True,
                    stop=True,
                )
                nc.scalar.activation(zst[:, co], pz, AF.Identity)
            nc.sync.dma_start(out=zT_r[:, :, qoff : qoff + TCH], in_=zst)

        # ---- emission: per batch, QKV chunks with the first q-chunk's
        # S.T/exp interleaved (ACT starts early), then the attention tails.
        for b in range(B):
            for c in range(NCH // B):
                emit_chunk(b * (NCH // B) + c)
                for kt in range(4 * c, 4 * c + 4):
                    emit_st_exp(b, 0, kt)
            emit_attention_tail(b, 0)
            for j in range(1, NSEQ // TCH):
                for kt in range(KT):
                    emit_st_exp(b, j, kt)
                emit_attention_tail(b, j)


def _build_nc(repeat=1, loop=True):
    _install_drain_patch()
    nc = bass.Bass("TRN2", target_bir_lowering=False, debug=False, num_devices=NCORES)
    xT = nc.dram_tensor("xT", [NIN, NTOK], BF16, kind="ExternalInput").ap()
    wq = nc.dram_tensor("wq", [NIN, P], BF16, kind="ExternalInput").ap()
    wk = nc.dram_tensor("wk", [NIN, P], BF16, kind="ExternalInput").ap()
    wv = nc.dram_tensor("wv", [NIN, P], BF16, kind="ExternalInput").ap()
    bqkv = nc.dram_tensor("bqkv", [P, 3], F32, kind="ExternalInput").ap()
    wo = nc.dram_tensor("wo", [P, NIN], BF16, kind="ExternalInput").ap()
    ones64 = nc.dram_tensor("ones64", [1, DH], F32R, kind="ExternalInput").ap()
    zT = nc.dram_tensor("zT", [NIN, NTOK], F32, kind="ExternalOutput").ap()
    with tile.TileContext(nc, num_cores=NCORES) as tc:
        if repeat == 1:
            _emit(tc, xT, wq, wk, wv, bqkv, wo, ones64, zT)
        elif loop:
            with tc.For_i(0, repeat):
                _emit(tc, xT, wq, wk, wv, bqkv, wo, ones64, zT)
        else:
            for _ in range(repeat):
                _emit(tc, xT, wq, wk, wv, bqkv, wo, ones64, zT)
    _split_multi_waits(nc)
    return nc


_NC_CACHE = None


def _get_nc():
    global _NC_CACHE
    if _NC_CACHE is None:
        _NC_CACHE = _build_nc()
    return _NC_CACHE


# -------------------------------------------------------------- host wrapper
def _to_bf16(a):
    import ml_dtypes

    return np.ascontiguousarray(a).astype(ml_dtypes.bfloat16)


def _in_maps(x, Wq, bq, Wk, bk, Wv, bv, Wo):
    xTh = _to_bf16(x.reshape(NTOK, NIN).T)
    maps = []
    for c in range(NCORES):
        F = slice(P * c, P * (c + 1))
        maps.append(
            {
                "xT": xTh,
                "wq": _to_bf16(Wq[F].T),
                "wk": _to_bf16(Wk[F].T),
                "wv": _to_bf16(Wv[F].T),
                "bqkv": np.ascontiguousarray(
                    np.stack([bq[F], bk[F], bv[F]], axis=1).astype(np.float32)
                ),
                "wo": _to_bf16(Wo[:, F].T),
                "ones64": np.ones((1, DH), np.float32),
            }
        )
    return maps


def kernel(x, Wq, bq, Wk, bk, Wv, bv, Wo, bo, **run_kwargs):
    x = np.asarray(x, np.float32)
    maps = _in_maps(
        x,
        np.asarray(Wq, np.float32),
        np.asarray(bq, np.float32),
        np.asarray(Wk, np.float32),
        np.asarray(bk, np.float32),
        np.asarray(Wv, np.float32),
        np.asarray(bv, np.float32),
        np.asarray(Wo, np.float32),
    )
    nc = _get_nc()
    res = run_bass_kernel_spmd(nc, maps, list(range(NCORES)), **run_kwargs)
    acc = res.results[0]["zT"].astype(np.float32)
    for c in range(1, NCORES):
        acc = acc + res.results[c]["zT"]
    z = acc.T + np.asarray(bo, np.float32)[None, :]
    out = np.ascontiguousarray(z.reshape(B, NSEQ, NIN), dtype=np.float32)
    if run_kwargs:
        return out, res
    return out


# revision 29
# speedup vs baseline: 1.4003x; 1.0170x over previous
"""Self-contained 8-NeuronCore Trainium2 kernel for a 16-head MHA layer.

Problem (hardcoded): x [2, 2048, 1024] f32, torch-style Linear weights
Wq/Wk/Wv/Wo [1024, 1024] + biases. y = MHA(x) with 16 heads of dim 64.

Sharding: tensor-parallel over heads. Core c owns heads {2c, 2c+1}, i.e.
feature slice F = [128c, 128c+128). Per core:

  phase 1   qT/kT/vT = W[F] @ x.T + b   (bf16 matmuls, feature-major)
            vT is PE-transposed per 128-token block into the PV lhsT
            layout v_pv[t, h] = [v|1] (bf16, ones column = softmax
            denominator rides as psum row 64 of the PV accumulation).
  phase 2   per (batch, q-chunk): dual-head S.T matmuls, exp split
            between ACT (exact, scale 1/8 fused) and DVE (Schraudolph
            int16-bitcast, SCHRA_NUM/16 of key tiles) for engine
            balance; PV accumulation; DVE reciprocal + K=1 matmul
            partition-broadcast normalize; per-chunk output projection,
            staged zT written with one batched DMA per 512-token chunk.
  engines   PSUM evacuations ride on ACT (Identity, cheaper per op);
            DVE does Schraudolph exp + reciprocal + normalize muls;
            attention for batch b's first q-chunk is emitted interleaved
            with phase-1 chunks so exp work starts ~6us into the kernel.

Host sums the 8 partial zT outputs, adds bo, transposes back.
"""

import numpy as np

import concourse.bass as bass
import concourse.tile as tile
from concourse import mybir
from concourse.bass_utils import run_bass_kernel_spmd
from concourse.masks import make_identity

# ---------------------------------------------------------------- constants
B = 2
NSEQ = 2048
NIN = 1024
H = 16
DH = 64
P = 128
NTOK = B * NSEQ            # 4096
KO = NIN // P              # 8 contraction chunks for the projections
NCORES = 8
TCH = 512                  # token chunk (psum bank, free dim)
QH = 1024                  # q columns per S.T psum tile / exp instruction
KT = NSEQ // P             # 16 key-token tiles per (batch, head)
NCH = NTOK // TCH          # 8 projection chunks
EXP_BUFS = 24              # live expS tiles

F32 = mybir.dt.float32
F32R = mybir.dt.float32r
BF16 = mybir.dt.bfloat16
I16 = mybir.dt.int16
AF = mybir.ActivationFunctionType
AL = mybir.AluOpType

_SCALE = 0.125             # 1/sqrt(DH)

# DVE Schraudolph exp: i16 = round(A*(s/8) + B) bitcast to bf16 ~= exp(s/8).
# B is centered (16256 - 7) so the +-3% mantissa-linearization ripple is
# zero-mean and mixes cleanly with exact-exp tiles in the same softmax.
_SCHRA_A = 184.6649652337873 * _SCALE
_SCHRA_B = 16249.0
# Fraction of key-tiles whose exp runs on DVE instead of ACT (engine
# balancing; error contribution validated numerically, see notes).
SCHRA_NUM = 10             # kt % 16 < SCHRA_NUM  ->  DVE Schraudolph


# ------------------------------------------------- walrus workaround (env)
# The walrus build in this environment rejects instructions carrying more
# than one semaphore wait ("Too many sync wait commands").
def _patched_drain_and_barrier(self, tick_clock, wait_clock):
    nc = self.nc
    drain_inst = nc.sync.drain()
    wait_clock.add_sem_waits(
        drain_inst.ins, tile.ScopedClock({None: tick_clock.global_clock})
    )
    si = drain_inst.ins.sync_info
    waits = list(si.on_wait) if si is not None else []
    if len(waits) > 1:
        drain_inst.ins.sync_info = mybir.SyncInfo(
            on_wait=[waits[0]], on_update=list(si.on_update)
        )
        for w in waits[1:]:
            extra = nc.sync.drain()
            extra.ins.sync_info = mybir.SyncInfo(on_wait=[w], on_update=[])
    nc.all_engine_barrier()
    popped = nc._tile_sem_poison_stack.pop()
    assert popped is self._sem_poison
    nc.clear_and_free_semaphores(list(self.sems.allocated().values()))
    nc.all_engine_barrier()


def _install_drain_patch():
    if getattr(tile.TileContext, "_drain_patch_installed", False):
        return
    tile.TileContext._drain_and_barrier = _patched_drain_and_barrier
    tile.TileContext._drain_patch_installed = True


def _split_multi_waits(nc):
    """Move extra semaphore waits onto same-engine NoOps placed immediately
    before the instruction (engines execute their stream in order)."""
    k = 0
    for f in nc.m.functions:
        for blk in f.blocks:
            lst = blk.instructions
            i = 0
            while i < len(lst):
                inst = lst[i]
                si = inst.sync_info
                waits = list(si.on_wait) if si is not None else []
                if len(waits) > 1:
                    for w in waits[:-1]:
                        nop = mybir.InstNoOp(
                            name=f"waitsplit-{k}", engine=inst.engine
                        )
                        k += 1
                        nop.sync_info = mybir.SyncInfo(on_wait=[w], on_update=[])
                        nc.register_instruction(nop)
                        lst.insert(i, nop)
                        i += 1
                    inst.sync_info = mybir.SyncInfo(
                        on_wait=[waits[-1]], on_update=list(si.on_update)
                    )
                i += 1
    return k


# ------------------------------------------------------------ device kernel
def _emit(tc, xT, wq, wk, wv, bqkv, wo, ones64, zT):
    nc = tc.nc

    xT_r = xT.rearrange("(ko ki) t -> ki ko t", ki=P)
    zT_r = zT.rearrange("(co ki) t -> ki co t", ki=P)

    with (
        tc.tile_pool(name="const", bufs=1) as const,
        tc.tile_pool(name="persist", bufs=1) as persist,
        tc.tile_pool(name="xin", bufs=2) as xin,
        tc.tile_pool(name="vtmp", bufs=2) as vtmp,
        tc.tile_pool(name="exps", bufs=EXP_BUFS) as exps,
        tc.tile_pool(name="work", bufs=4) as work,
        tc.tile_pool(name="zout", bufs=2) as zout,
        tc.tile_pool(name="ps_big", bufs=2, space="PSUM") as ps_big,
        tc.tile_pool(name="ps_pv", bufs=2, space="PSUM") as ps_pv,
        tc.tile_pool(name="ps_mm", bufs=2, space="PSUM") as ps_mm,
    ):
        # ---- constants
        wq_sb = const.tile([P, KO, P], BF16)
        wk_sb = const.tile([P, KO, P], BF16)
        wv_sb = const.tile([P, KO, P], BF16)
        for w_sb, w in ((wq_sb, wq), (wk_sb, wk), (wv_sb, wv)):
            nc.sync.dma_start(out=w_sb, in_=w.rearrange("(ko ki) m -> ki ko m", ki=P))
        wo_sb = const.tile([P, NIN], BF16)
        nc.sync.dma_start(out=wo_sb, in_=wo)
        bias_sb = const.tile([P, 3], F32)
        nc.sync.dma_start(out=bias_sb, in_=bqkv)
        ones_sb = const.tile([1, DH], F32R)
        nc.sync.dma_start(out=ones_sb, in_=ones64)
        ident = const.tile([P, P], BF16)
        make_identity(nc, ident)

        qT = persist.tile([P, NTOK], BF16)
        kT = persist.tile([P, NTOK], BF16)
        # PV lhsT per 128-token block, per head slot: [v(64) | 1]; the ones
        # column makes the softmax denominator ride as psum row 64.
        v_pv = persist.tile([P, NTOK // P, 2, DH + 1], BF16)
        nc.vector.memset(v_pv, 1.0)
        OT = persist.tile([P, NTOK], BF16)

        etiles = {}

        def emit_st_exp(b, j, kt):
            """Dual-head S.T matmul pair for one 128-key tile + fused exp.
            exp runs on ACT (exact) or DVE (Schraudolph int16 bitcast) so the
            two engines split the softmax elementwise load."""
            qoff = b * NSEQ + j * TCH
            ps = ps_big.tile([P, QH], F32, tag="st")
            ksl = slice(b * NSEQ + kt * P, b * NSEQ + (kt + 1) * P)
            for hl in range(2):
                hsl = slice(DH * hl, DH * hl + DH)
                nc.tensor.matmul(
                    ps[:, hl * TCH : (hl + 1) * TCH],
                    kT[hsl, ksl],
                    qT[hsl, qoff : qoff + TCH],
                    start=True,
                    stop=True,
                )
            if kt % KT < SCHRA_NUM:
                ei = exps.tile([P, QH], I16, tag="e")
                nc.vector.tensor_scalar(
                    out=ei, in0=ps, scalar1=_SCHRA_A, scalar2=_SCHRA_B,
                    op0=AL.mult, op1=AL.add,
                )
                e = ei.bitcast(BF16)
            else:
                e = exps.tile([P, QH], BF16, tag="e")
                nc.scalar.activation(e, ps, AF.Exp, scale=_SCALE)
            etiles[(b, j, kt)] = e

        def emit_chunk(n):
            """QKV projections for one 512-token chunk (feature-major)."""
            tsl = slice(n * TCH, (n + 1) * TCH)
            xt = xin.tile([P, KO, TCH], BF16, tag="xt")
            nc.sync.dma_start(out=xt, in_=xT_r[:, :, tsl])
            # q, k  (evacuation + bias on ACT; DVE is reserved for exp work)
            for pi, (w_sb, dst) in enumerate(((wq_sb, qT), (wk_sb, kT))):
                ps = ps_mm.tile([P, TCH], F32, tag="mm")
                for ko in range(KO):
                    nc.tensor.matmul(
                        ps, w_sb[:, ko], xt[:, ko],
                        start=(ko == 0), stop=(ko == KO - 1),
                    )
                nc.scalar.activation(
                    dst[:, tsl], ps, AF.Identity,
                    bias=bias_sb[:, pi : pi + 1],
                )
            # v: feature-major matmul + bias, then PE transpose per
            # 128-token block into the packed PV lhsT layout.
            ps = ps_mm.tile([P, TCH], F32, tag="mm")
            for ko in range(KO):
                nc.tensor.matmul(
                    ps, wv_sb[:, ko], xt[:, ko],
                    start=(ko == 0), stop=(ko == KO - 1),
                )
            vt = vtmp.tile([P, TCH], BF16, tag="vt")
            nc.scalar.activation(vt, ps, AF.Identity, bias=bias_sb[:, 2:3])
            pst = ps_mm.tile([P, TCH], BF16, tag="mm")
            for t2 in range(TCH // P):
                nc.tensor.matmul(
                    pst[:, t2 * P : (t2 + 1) * P],
                    vt[:, t2 * P : (t2 + 1) * P],
                    ident,
                    is_transpose=True,
                    start=True,
                    stop=True,
                )
            pst_v = pst.rearrange("p (t2 h f) -> p t2 h f", t2=TCH // P, h=2)
            t0 = n * (TCH // P)
            nc.scalar.activation(
                v_pv[:, t0 : t0 + TCH // P, :, 0:DH], pst_v, AF.Identity
            )

        def emit_attention_tail(b, j):
            """PV + normalize + output projection for one (batch, q-chunk).
            Assumes S.T/exp for (b, j, *) already emitted."""
            qoff = b * NSEQ + j * TCH
            dst = slice(qoff, qoff + TCH)
            pvps = {}
            for hl in (1, 0):
                pvps[hl] = ps_pv.tile(
                    [P, TCH], F32, tag="pv", name=f"pv_{b}_{j}_{hl}"
                )
            # Both heads accumulate on psum rows 0..64 ([v|1], denominator
            # at row 64); interleaved so the two normalize chains overlap.
            for kt in range(KT):
                e = etiles[(b, j, kt)]
                for hl in (1, 0):
                    nc.tensor.matmul(
                        pvps[hl][0 : DH + 1, :],
                        v_pv[:, b * KT + kt, hl],
                        e[:, hl * TCH : (hl + 1) * TCH],
                        start=(kt == 0),
                        stop=(kt == KT - 1),
                    )
            for hl in (1, 0):      # h1 first: its OT shift-DMA overlaps h0
                ps = pvps[hl]
                rec = work.tile([1, TCH], F32R, tag="rec")
                with nc.allow_low_precision(
                    reason="f32r is bit-identical to f32; PE rounds on read"
                ):
                    nc.vector.reciprocal(rec, ps[DH : DH + 1, :])
                # partition-broadcast via K=1 matmul
                psb = ps_mm.tile([P, TCH], F32, tag="mm")
                nc.tensor.matmul(
                    psb[0:DH, :], ones_sb, rec, start=True, stop=True
                )
                recB = work.tile([DH, TCH], F32, tag="recB")
                nc.scalar.activation(recB, psb[0:DH, :], AF.Identity)
                if hl == 0:
                    nc.vector.tensor_mul(OT[0:DH, dst], ps[0:DH, :], recB)
                else:
                    tmpO = work.tile([DH, TCH], BF16, tag="tmpO")
                    nc.vector.tensor_mul(tmpO, ps[0:DH, :], recB)
                    nc.sync.dma_start(out=OT[DH:P, dst], in_=tmpO)
            for kt in range(KT):
                del etiles[(b, j, kt)]
            # ---- output projection for this 512-token chunk
            zst = zout.tile([P, NIN // P, TCH], F32, tag="z")
            for co in range(NIN // P):
                pz = ps_mm.tile([P, TCH], F32, tag="mm")
                nc.tensor.matmul(
                    pz,
                    wo_sb[:, co * P : (co + 1) * P],
                    OT[:, qoff : qoff + TCH],
                    start=True,
                    stop=True,
                )
                nc.scalar.activation(zst[:, co], pz, AF.Identity)
            nc.sync.dma_start(out=zT_r[:, :, qoff : qoff + TCH], in_=zst)

        # ---- emission: per batch, QKV chunks with the first q-chunk's
        # S.T/exp interleaved (ACT starts early), then the attention tails.
        for b in range(B):
            for c in range(NCH // B):
                emit_chunk(b * (NCH // B) + c)
                for kt in range(4 * c, 4 * c + 4):
                    emit_st_exp(b, 0, kt)
            emit_attention_tail(b, 0)
            for j in range(1, NSEQ // TCH):
                for kt in range(KT):
                    emit_st_exp(b, j, kt)
                emit_attention_tail(b, j)


def _build_nc(repeat=1, loop=True):
    _install_drain_patch()
    nc = bass.Bass("TRN2", target_bir_lowering=False, debug=False, num_devices=NCORES)
    xT = nc.dram_tensor("xT", [NIN, NTOK], BF16, kind="ExternalInput").ap()
    wq = nc.dram_tensor("wq", [NIN, P], BF16, kind="ExternalInput").ap()
    wk = nc.dram_tensor("wk", [NIN, P], BF16, kind="ExternalInput").ap()
    wv = nc.dram_tensor("wv", [NIN, P], BF16, kind="ExternalInput").ap()
    bqkv = nc.dram_tensor("bqkv", [P, 3], F32, kind="ExternalInput").ap()
    wo = nc.dram_tensor("wo", [P, NIN], BF16, kind="ExternalInput").ap()
    ones64 = nc.dram_tensor("ones64", [1, DH], F32R, kind="ExternalInput").ap()
    zT = nc.dram_tensor("zT", [NIN, NTOK], F32, kind="ExternalOutput").ap()
    with tile.TileContext(nc, num_cores=NCORES) as tc:
        if repeat == 1:
            _emit(tc, xT, wq, wk, wv, bqkv, wo, ones64, zT)
        elif loop:
            with tc.For_i(0, repeat):
                _emit(tc, xT, wq, wk, wv, bqkv, wo, ones64, zT)
        else:
            for _ in range(repeat):
                _emit(tc, xT, wq, wk, wv, bqkv, wo, ones64, zT)
    _split_multi_waits(nc)
    return nc


_NC_CACHE = None


def _get_nc():
    global _NC_CACHE
    if _NC_CACHE is None:
        _NC_CACHE = _build_nc()
    return _NC_CACHE


# -------------------------------------------------------------- host wrapper
def _to_bf16(a):
    import ml_dtypes

    return np.ascontiguousarray(a).astype(ml_dtypes.bfloat16)


def _in_maps(x, Wq, bq, Wk, bk, Wv, bv, Wo):
    xTh = _to_bf16(x.reshape(NTOK, NIN).T)
    maps = []
    for c in range(NCORES):
        F = slice(P * c, P * (c + 1))
        maps.append(
            {
                "xT": xTh,
                "wq": _to_bf16(Wq[F].T),
                "wk": _to_bf16(Wk[F].T),
                "wv": _to_bf16(Wv[F].T),
                "bqkv": np.ascontiguousarray(
                    np.stack([bq[F], bk[F], bv[F]], axis=1).astype(np.float32)
                ),
                "wo": _to_bf16(Wo[:, F].T),
                "ones64": np.ones((1, DH), np.float32),
            }
        )
    return maps


def kernel(x, Wq, bq, Wk, bk, Wv, bv, Wo, bo, **run_kwargs):
    x = np.asarray(x, np.float32)
    maps = _in_maps(
        x,
        np.asarray(Wq, np.float32),
        np.asarray(bq, np.float32),
        np.asarray(Wk, np.float32),
        np.asarray(bk, np.float32),
        np.asarray(Wv, np.float32),
        np.asarray(bv, np.float32),
        np.asarray(Wo, np.float32),
    )
    nc = _get_nc()
    res = run_bass_kernel_spmd(nc, maps, list(range(NCORES)), **run_kwargs)
    acc = res.results[0]["zT"].astype(np.float32)
    for c in range(1, NCORES):
        acc = acc + res.results[c]["zT"]
    z = acc.T + np.asarray(bo, np.float32)[None, :]
    out = np.ascontiguousarray(z.reshape(B, NSEQ, NIN), dtype=np.float32)
    if run_kwargs:
        return out, res
    return out


# revision 35
# speedup vs baseline: 1.8211x; 1.3005x over previous
"""Self-contained 8-NeuronCore Trainium2 kernel for a 16-head MHA layer.

Problem (hardcoded): x [2, 2048, 1024] f32, torch-style Linear weights
Wq/Wk/Wv/Wo [1024, 1024] + biases. y = MHA(x) with 16 heads of dim 64.

Sharding: tensor-parallel over heads. Core c owns heads {2c, 2c+1}, i.e.
feature slice F = [128c, 128c+128). Per core:

  phase 1   qT/kT/vT = W[F] @ x.T + b   (bf16 matmuls, feature-major)
            vT is PE-transposed per 128-token block into the PV lhsT
            layout v_pv[t, h] = [v|1] (bf16, ones column = softmax
            denominator rides as psum row 64 of the PV accumulation).
  phase 2   per (batch, q-chunk): dual-head S.T matmuls, exp split
            between ACT (exact, scale 1/8 fused) and DVE (Schraudolph
            int16-bitcast, SCHRA_NUM/16 of key tiles) for engine
            balance; PV accumulation; DVE reciprocal + K=1 matmul
            partition-broadcast normalize; per-chunk output projection,
            staged zT written with one batched DMA per 512-token chunk.
  engines   PSUM evacuations ride on ACT (Identity, cheaper per op);
            DVE does Schraudolph exp + reciprocal + normalize muls;
            attention for batch b's first q-chunk is emitted interleaved
            with phase-1 chunks so exp work starts ~6us into the kernel.

Host sums the 8 partial zT outputs, adds bo, transposes back.
"""

import numpy as np

import concourse.bass as bass
import concourse.tile as tile
from concourse import mybir
from concourse.bass_utils import run_bass_kernel_spmd
from concourse.masks import make_identity

# ---------------------------------------------------------------- constants
B = 2
NSEQ = 2048
NIN = 1024
H = 16
DH = 64
P = 128
NTOK = B * NSEQ            # 4096
KO = NIN // P              # 8 contraction chunks for the projections
NCORES = 8
TCH = 512                  # token chunk (psum bank, free dim)
QH = 1024                  # q columns per S.T psum tile / exp instruction
KT = NSEQ // P             # 16 key-token tiles per (batch, head)
NCH = NTOK // TCH          # 8 projection chunks
EXP_BUFS = 24              # live expS tiles

F32 = mybir.dt.float32
F32R = mybir.dt.float32r
BF16 = mybir.dt.bfloat16
I16 = mybir.dt.int16
AF = mybir.ActivationFunctionType
AL = mybir.AluOpType

_SCALE = 0.125             # 1/sqrt(DH)

# DVE Schraudolph exp: i16 = round(A*(s/8) + B) bitcast to bf16 ~= exp(s/8).
# B is centered (16256 - 7) so the +-3% mantissa-linearization ripple is
# zero-mean and mixes cleanly with exact-exp tiles in the same softmax.
_SCHRA_A = 184.6649652337873 * _SCALE
_SCHRA_B = 16249.0
# Key-tiles whose exp runs on DVE instead of ACT (10 of 16, alternating so
# both consumer engines drain the S.T psum tiles concurrently; error
# contribution validated numerically, see notes).
SCHRA_SET = frozenset((0, 1, 3, 4, 6, 8, 9, 11, 12, 14))


# ------------------------------------------------- walrus workaround (env)
# The walrus build in this environment rejects instructions carrying more
# than one semaphore wait ("Too many sync wait commands").
def _patched_drain_and_barrier(self, tick_clock, wait_clock):
    nc = self.nc
    drain_inst = nc.sync.drain()
    wait_clock.add_sem_waits(
        drain_inst.ins, tile.ScopedClock({None: tick_clock.global_clock})
    )
    si = drain_inst.ins.sync_info
    waits = list(si.on_wait) if si is not None else []
    if len(waits) > 1:
        drain_inst.ins.sync_info = mybir.SyncInfo(
            on_wait=[waits[0]], on_update=list(si.on_update)
        )
        for w in waits[1:]:
            extra = nc.sync.drain()
            extra.ins.sync_info = mybir.SyncInfo(on_wait=[w], on_update=[])
    nc.all_engine_barrier()
    popped = nc._tile_sem_poison_stack.pop()
    assert popped is self._sem_poison
    nc.clear_and_free_semaphores(list(self.sems.allocated().values()))
    nc.all_engine_barrier()


def _install_drain_patch():
    if getattr(tile.TileContext, "_drain_patch_installed", False):
        return
    tile.TileContext._drain_and_barrier = _patched_drain_and_barrier
    tile.TileContext._drain_patch_installed = True


def _split_multi_waits(nc):
    """Move extra semaphore waits onto same-engine NoOps placed immediately
    before the instruction (engines execute their stream in order)."""
    k = 0
    for f in nc.m.functions:
        for blk in f.blocks:
            lst = blk.instructions
            i = 0
            while i < len(lst):
                inst = lst[i]
                si = inst.sync_info
                waits = list(si.on_wait) if si is not None else []
                if len(waits) > 1:
                    for w in waits[:-1]:
                        nop = mybir.InstNoOp(
                            name=f"waitsplit-{k}", engine=inst.engine
                        )
                        k += 1
                        nop.sync_info = mybir.SyncInfo(on_wait=[w], on_update=[])
                        nc.register_instruction(nop)
                        lst.insert(i, nop)
                        i += 1
                    inst.sync_info = mybir.SyncInfo(
                        on_wait=[waits[-1]], on_update=list(si.on_update)
                    )
                i += 1
    return k


# ------------------------------------------------------------ device kernel
def _emit(tc, xT, wq, wk, wv, bqkv, wo, ones64, zT):
    nc = tc.nc

    xT_r = xT.rearrange("(ko ki) t -> ki ko t", ki=P)
    zT_r = zT.rearrange("(co ki) t -> ki co t", ki=P)

    with (
        tc.tile_pool(name="const", bufs=1) as const,
        tc.tile_pool(name="persist", bufs=1) as persist,
        tc.tile_pool(name="xin", bufs=2) as xin,
        tc.tile_pool(name="vtmp", bufs=2) as vtmp,
        tc.tile_pool(name="exps", bufs=EXP_BUFS) as exps,
        tc.tile_pool(name="work", bufs=4) as work,
        tc.tile_pool(name="zout", bufs=2) as zout,
        tc.tile_pool(name="ps_big", bufs=2, space="PSUM") as ps_big,
        tc.tile_pool(name="ps_pv", bufs=2, space="PSUM") as ps_pv,
        tc.tile_pool(name="ps_mm", bufs=2, space="PSUM") as ps_mm,
    ):
        # ---- constants
        wq_sb = const.tile([P, KO, P], BF16)
        wk_sb = const.tile([P, KO, P], BF16)
        wv_sb = const.tile([P, KO, P], BF16)
        for w_sb, w in ((wq_sb, wq), (wk_sb, wk), (wv_sb, wv)):
            nc.sync.dma_start(out=w_sb, in_=w.rearrange("(ko ki) m -> ki ko m", ki=P))
        wo_sb = const.tile([P, NIN], BF16)
        nc.sync.dma_start(out=wo_sb, in_=wo)
        bias_sb = const.tile([P, 3], F32)
        nc.sync.dma_start(out=bias_sb, in_=bqkv)
        ones_sb = const.tile([1, DH], F32R)
        nc.sync.dma_start(out=ones_sb, in_=ones64)
        ident = const.tile([P, P], BF16)
        make_identity(nc, ident)

        qT = persist.tile([P, NTOK], BF16)
        kT = persist.tile([P, NTOK], BF16)
        # PV lhsT per 128-token block, per head slot: [v(64) | 1]; the ones
        # column makes the softmax denominator ride as psum row 64.
        v_pv = persist.tile([P, NTOK // P, 2, DH + 1], BF16)
        nc.vector.memset(v_pv, 1.0)
        OT = persist.tile([P, NTOK], BF16)

        etiles = {}

        def emit_st_exp(b, j, kt):
            """Dual-head S.T matmul pair for one 128-key tile + fused exp.
            exp runs on ACT (exact) or DVE (Schraudolph int16 bitcast) so the
            two engines split the softmax elementwise load."""
            qoff = b * NSEQ + j * TCH
            ps = ps_big.tile([P, QH], F32, tag="st")
            ksl = slice(b * NSEQ + kt * P, b * NSEQ + (kt + 1) * P)
            for hl in range(2):
                hsl = slice(DH * hl, DH * hl + DH)
                nc.tensor.matmul(
                    ps[:, hl * TCH : (hl + 1) * TCH],
                    kT[hsl, ksl],
                    qT[hsl, qoff : qoff + TCH],
                    start=True,
                    stop=True,
                )
            if kt % KT in SCHRA_SET:
                ei = exps.tile([P, QH], I16, tag="e")
                nc.vector.tensor_scalar(
                    out=ei, in0=ps, scalar1=_SCHRA_A, scalar2=_SCHRA_B,
                    op0=AL.mult, op1=AL.add,
                )
                e = ei.bitcast(BF16)
            else:
                e = exps.tile([P, QH], BF16, tag="e")
                nc.scalar.activation(e, ps, AF.Exp, scale=_SCALE)
            etiles[(b, j, kt)] = e

        def emit_chunk(n):
            """QKV projections for one 512-token chunk (feature-major)."""
            tsl = slice(n * TCH, (n + 1) * TCH)
            xt = xin.tile([P, KO, TCH], BF16, tag="xt")
            nc.sync.dma_start(out=xt, in_=xT_r[:, :, tsl])
            # q, k  (evacuation + bias on ACT; DVE is reserved for exp work)
            for pi, (w_sb, dst) in enumerate(((wq_sb, qT), (wk_sb, kT))):
                ps = ps_mm.tile([P, TCH], F32, tag="mm")
                for ko in range(KO):
                    nc.tensor.matmul(
                        ps, w_sb[:, ko], xt[:, ko],
                        start=(ko == 0), stop=(ko == KO - 1),
                    )
                nc.scalar.activation(
                    dst[:, tsl], ps, AF.Identity,
                    bias=bias_sb[:, pi : pi + 1],
                )
            # v: feature-major matmul + bias, then PE transpose per
            # 128-token block into the packed PV lhsT layout.
            ps = ps_mm.tile([P, TCH], F32, tag="mm")
            for ko in range(KO):
                nc.tensor.matmul(
                    ps, wv_sb[:, ko], xt[:, ko],
                    start=(ko == 0), stop=(ko == KO - 1),
                )
            vt = vtmp.tile([P, TCH], BF16, tag="vt")
            nc.scalar.activation(vt, ps, AF.Identity, bias=bias_sb[:, 2:3])
            pst = ps_mm.tile([P, TCH], BF16, tag="mm")
            for t2 in range(TCH // P):
                nc.tensor.matmul(
                    pst[:, t2 * P : (t2 + 1) * P],
                    vt[:, t2 * P : (t2 + 1) * P],
                    ident,
                    is_transpose=True,
                    start=True,
                    stop=True,
                )
            pst_v = pst.rearrange("p (t2 h f) -> p t2 h f", t2=TCH // P, h=2)
            t0 = n * (TCH // P)
            nc.scalar.activation(
                v_pv[:, t0 : t0 + TCH // P, :, 0:DH], pst_v, AF.Identity
            )

        pvstate = {}

        def emit_pv_pair(b, j, kt):
            """One key-tile's PV accumulation for both heads of (b, j).
            Emitted interleaved between the NEXT q-chunk's S.T matmuls so
            the PE has exp-independent work while ACT/DVE drain exps."""
            if kt == 0:
                pvstate[(b, j)] = {
                    hl: ps_pv.tile(
                        [P, TCH], F32, tag="pv", name=f"pv_{b}_{j}_{hl}"
                    )
                    for hl in (1, 0)
                }
            pvps = pvstate[(b, j)]
            e = etiles[(b, j, kt)]
            # Both heads accumulate on psum rows 0..64 ([v|1], denominator
            # at row 64); interleaved so the two normalize chains overlap.
            for hl in (1, 0):
                nc.tensor.matmul(
                    pvps[hl][0 : DH + 1, :],
                    v_pv[:, b * KT + kt, hl],
                    e[:, hl * TCH : (hl + 1) * TCH],
                    start=(kt == 0),
                    stop=(kt == KT - 1),
                )
            del etiles[(b, j, kt)]

        def emit_finish(b, j):
            """Normalize + output projection for one (batch, q-chunk);
            assumes all 16 PV pairs for (b, j) already emitted."""
            qoff = b * NSEQ + j * TCH
            dst = slice(qoff, qoff + TCH)
            pvps = pvstate.pop((b, j))
            for hl in (1, 0):      # h1 first: its OT shift-DMA overlaps h0
                ps = pvps[hl]
                rec = work.tile([1, TCH], F32R, tag="rec")
                with nc.allow_low_precision(
                    reason="f32r is bit-identical to f32; PE rounds on read"
                ):
                    nc.vector.reciprocal(rec, ps[DH : DH + 1, :])
                # partition-broadcast via K=1 matmul
                psb = ps_mm.tile([P, TCH], F32, tag="mm")
                nc.tensor.matmul(
                    psb[0:DH, :], ones_sb, rec, start=True, stop=True
                )
                recB = work.tile([DH, TCH], F32, tag="recB")
                nc.scalar.activation(recB, psb[0:DH, :], AF.Identity)
                if hl == 0:
                    nc.vector.tensor_mul(OT[0:DH, dst], ps[0:DH, :], recB)
                else:
                    tmpO = work.tile([DH, TCH], BF16, tag="tmpO")
                    nc.vector.tensor_mul(tmpO, ps[0:DH, :], recB)
                    nc.sync.dma_start(out=OT[DH:P, dst], in_=tmpO)
            # ---- output projection for this 512-token chunk
            zst = zout.tile([P, NIN // P, TCH], F32, tag="z")
            for co in range(NIN // P):
                pz = ps_mm.tile([P, TCH], F32, tag="mm")
                nc.tensor.matmul(
                    pz,
                    wo_sb[:, co * P : (co + 1) * P],
                    OT[:, qoff : qoff + TCH],
                    start=True,
                    stop=True,
                )
                nc.scalar.activation(zst[:, co], pz, AF.Identity)
            nc.sync.dma_start(out=zT_r[:, :, qoff : qoff + TCH], in_=zst)

        # ---- emission: software-pipelined two stages deep.  Each q-chunk's
        # 16 S.T matmul slots carry the PREVIOUS q-chunk's 16 PV pairs
        # between them, so the PE always has exp-independent work while
        # ACT/DVE drain the new S.T psum tiles.  The first q-chunk of each
        # batch is interleaved into the phase-1 QKV chunks (exp starts
        # early); its PV rides the next batch's chunk loop.
        pending = None
        for b in range(B):
            for c in range(NCH // B):
                emit_chunk(b * (NCH // B) + c)
                for kt in range(4 * c, 4 * c + 4):
                    emit_st_exp(b, 0, kt)
                    if pending is not None:
                        emit_pv_pair(*pending, kt)
            if pending is not None:
                emit_finish(*pending)
            pending = (b, 0)
            for j in range(1, NSEQ // TCH):
                for kt in range(KT):
                    emit_st_exp(b, j, kt)
                    emit_pv_pair(*pending, kt)
                emit_finish(*pending)
                pending = (b, j)
        # drain the last q-chunk's PV (no S.T stream left to pair with)
        for kt in range(KT):
            emit_pv_pair(*pending, kt)
        emit_finish(*pending)


def _build_nc(repeat=1, loop=True):
    _install_drain_patch()
    nc = bass.Bass("TRN2", target_bir_lowering=False, debug=False, num_devices=NCORES)
    xT = nc.dram_tensor("xT", [NIN, NTOK], BF16, kind="ExternalInput").ap()
    wq = nc.dram_tensor("wq", [NIN, P], BF16, kind="ExternalInput").ap()
    wk = nc.dram_tensor("wk", [NIN, P], BF16, kind="ExternalInput").ap()
    wv = nc.dram_tensor("wv", [NIN, P], BF16, kind="ExternalInput").ap()
    bqkv = nc.dram_tensor("bqkv", [P, 3], F32, kind="ExternalInput").ap()
    wo = nc.dram_tensor("wo", [P, NIN], BF16, kind="ExternalInput").ap()
    ones64 = nc.dram_tensor("ones64", [1, DH], F32R, kind="ExternalInput").ap()
    zT = nc.dram_tensor("zT", [NIN, NTOK], F32, kind="ExternalOutput").ap()
    with tile.TileContext(nc, num_cores=NCORES) as tc:
        if repeat == 1:
            _emit(tc, xT, wq, wk, wv, bqkv, wo, ones64, zT)
        elif loop:
            with tc.For_i(0, repeat):
                _emit(tc, xT, wq, wk, wv, bqkv, wo, ones64, zT)
        else:
            for _ in range(repeat):
                _emit(tc, xT, wq, wk, wv, bqkv, wo, ones64, zT)
    _split_multi_waits(nc)
    return nc


_NC_CACHE = None


def _get_nc():
    global _NC_CACHE
    if _NC_CACHE is None:
        _NC_CACHE = _build_nc()
    return _NC_CACHE


# -------------------------------------------------------------- host wrapper
def _to_bf16(a):
    import ml_dtypes

    return np.ascontiguousarray(a).astype(ml_dtypes.bfloat16)


def _in_maps(x, Wq, bq, Wk, bk, Wv, bv, Wo):
    xTh = _to_bf16(x.reshape(NTOK, NIN).T)
    maps = []
    for c in range(NCORES):
        F = slice(P * c, P * (c + 1))
        maps.append(
            {
                "xT": xTh,
                "wq": _to_bf16(Wq[F].T),
                "wk": _to_bf16(Wk[F].T),
                "wv": _to_bf16(Wv[F].T),
                "bqkv": np.ascontiguousarray(
                    np.stack([bq[F], bk[F], bv[F]], axis=1).astype(np.float32)
                ),
                "wo": _to_bf16(Wo[:, F].T),
                "ones64": np.ones((1, DH), np.float32),
            }
        )
    return maps


def kernel(x, Wq, bq, Wk, bk, Wv, bv, Wo, bo, **run_kwargs):
    x = np.asarray(x, np.float32)
    maps = _in_maps(
        x,
        np.asarray(Wq, np.float32),
        np.asarray(bq, np.float32),
        np.asarray(Wk, np.float32),
        np.asarray(bk, np.float32),
        np.asarray(Wv, np.float32),
        np.asarray(bv, np.float32),
        np.asarray(Wo, np.float32),
    )
    nc = _get_nc()
    res = run_bass_kernel_spmd(nc, maps, list(range(NCORES)), **run_kwargs)
    acc = res.results[0]["zT"].astype(np.float32)
    for c in range(1, NCORES):
        acc = acc + res.results[c]["zT"]
    z = acc.T + np.asarray(bo, np.float32)[None, :]
    out = np.ascontiguousarray(z.reshape(B, NSEQ, NIN), dtype=np.float32)
    if run_kwargs:
        return out, res
    return out
